# revision 31
# baseline (speedup 1.0000x reference)
"""Trainium2 Bass kernel for nn_NonLocalDenoiser (LIDIA Aggregation0, top-1 self
neighbor): weighted patch fold -> normalize -> unfold, per pseudo-frame.

Shapes (hardcoded): x (2, 24336, 14, 75), nlDists (28, 24336, 14),
nlInds (28, 24336, 14, 3), H=W=160, PS=5, C=3.

Sharding: t=28 frames, each split into top/bottom half-slabs (82 input patch
rows with 4-row halo, 78 output rows); bottom slabs are row+dy flipped so all
56 tasks are identical. 7 tasks per core across 8 cores.

Device pipeline per task:
  - DMA x-slab into a zero-padded SBUF canvas (100 blocks of 160 cols:
    75 feature planes + 25 exp(-d) weight planes, 4-col leading pad)
  - ACT: w = exp(-dist) replicated into the 25 weight planes
  - DVE: in-place multiply feature planes by w
  - PE: fold = 2x25 matmuls with shifted-identity weights accumulating the
    (y, {c0,c1,c2,wimg}, x) image canvas in PSUM; column shifts read into the
    zero padding so every matmul writes the identical PSUM AP
  - DVE: rimg = 1/wimg; nimg = img * rimg  (PSUM -> SBUF)
  - DMA unfold: 5 strided reads of nimg -> HBM (dy-major output layout)
"""
import numpy as np

PS, C, NH, W = 5, 3, 156, 160
RIN, ROUT, HORF, VF = 82, 78, 14, 75
NT = 7            # tasks per core
NCORES = 8
T = 28            # pseudo-frames
NPATCH = NH * NH
NBLK = 4 * 25     # sbuf canvas blocks: (c0,c1,c2,w) x 25 (dy,dx)
# canvas: 100 blocks of [4-col zero pad | 156 data cols]; +4 tail cols so the
# last block's dx-overflow reads stay in-bounds. Feature blocks 0..74 are
# shipped pre-padded from the host (fully contiguous in-DMA).
PITCH = NBLK * W + 4   # 16004 floats per partition

# v index permutation for bottom (row-flipped) tasks: (c,dy,dx) -> (c,4-dy,dx)
VPERM = np.array([c * 25 + (4 - dy) * 5 + dx
                  for c in range(C) for dy in range(PS) for dx in range(PS)])

LAST_EXEC_NS = None


def _build_program(loop_reps=1, do_out=True, do_mm=True, do_tt=True):
    import contextlib
    import concourse.bass as bass
    import concourse.bacc as bacc
    import concourse.mybir as mybir
    import concourse.tile as tile

    f32 = mybir.dt.float32
    nc = bacc.Bacc(None)
    XS = nc.declare_dram_parameter("xs", [NT, RIN, VF * W], f32, isOutput=False)
    DS = nc.declare_dram_parameter("ds", [NT, RIN, NH], f32, isOutput=False)
    OUT = nc.declare_dram_parameter("out", [NT, PS, C, ROUT, PS, NH], f32,
                                    isOutput=True)
    M = RIN + 4  # img rows per slab (86)

    with tile.TileContext(nc) as tc:
        with tc.tile_pool(name="const", bufs=1) as cpool, \
             tc.tile_pool(name="xsp", bufs=2) as xpool, \
             tc.tile_pool(name="dp", bufs=2) as dpool, \
             tc.tile_pool(name="im", bufs=2) as ipool, \
             tc.tile_pool(name="ps", bufs=2, space="PSUM") as ppool:
            # 5 shifted identities: ids_dy[hi, y] = 1 iff y == hi + dy
            ids = cpool.tile([RIN, PS * M], f32)
            nc.gpsimd.memset(ids[:], 0.0)
            for dy in range(PS):
                sl = ids[:, dy * M:(dy + 1) * M]
                nc.gpsimd.affine_select(
                    out=sl, in_=sl, pattern=[[-1, M]],
                    compare_op=mybir.AluOpType.not_equal, fill=1.0,
                    base=dy, channel_multiplier=1)

            def stage_front(j):
                xs_t = xpool.tile([RIN, PITCH], f32, tag="xs")
                d_t = dpool.tile([RIN, NH], f32, tag="d")
                full = xs_t[:]
                pitch = full.ap[0][0]
                # zero the w-region block pads + tail (feature-block pads are
                # shipped as zeros from the host)
                pad_ap = bass.AP(full.tensor, full.offset + 75 * W,
                                 [[pitch, RIN], [W, 26], [1, 4]])
                nc.scalar.memzero(pad_ap)
                # contiguous feature-region load (75 pre-padded blocks)
                nc.sync.dma_start(out=xs_t[:, 0:VF * W], in_=XS[j])
                nc.sync.dma_start(out=d_t[:], in_=DS[j])
                # weight planes: w = exp(-d), replicated 25x
                data4 = xs_t[:, 0:NBLK * W].rearrange("p (c v q) -> p c v q",
                                                      c=4, v=25)
                nc.scalar.activation(
                    out=data4[:, 3:4, :, 4:W].squeeze(1),
                    in_=d_t[:].unsqueeze(1).to_broadcast([RIN, 25, NH]),
                    func=mybir.ActivationFunctionType.Exp, scale=-1.0)
                # feature planes *= w: one 2D TT per channel over the whole
                # c-block (incl. zero pads: 0 * w_pad = 0 keeps them zero)
                ch = xs_t[:, 0:NBLK * W].rearrange("p (c q) -> p c q", q=25 * W)
                for c in range(C if do_tt else 0):
                    nc.vector.tensor_tensor(
                        out=ch[:, c, :], in0=ch[:, c, :], in1=ch[:, 3, :],
                        op=mybir.AluOpType.mult)

                # fold: psA = (c0,c1) image canvas, psB = (c2, wimg)
                psA = ppool.tile([M, 2 * W], f32, tag="psA", space="PSUM")
                psB = ppool.tile([M, 2 * W], f32, tag="psB", space="PSUM")
                blocks = xs_t[:, 0:NBLK * W].rearrange("p (b q) -> p b q", q=W)
                nv0 = 25 if do_mm else 1
                for ps_t, cb in ((psA, 0), (psB, 2)):
                    for dy in range(PS):
                        lhsT = ids[:, dy * M:(dy + 1) * M]
                        for dx in range(PS):
                            v0 = dy * PS + dx
                            if v0 >= nv0:
                                continue
                            r = blocks[:, cb * 25 + v0: cb * 25 + v0 + 26: 25, :]
                            rs = bass.AP(r.tensor, r.offset + 4 - dx, r.ap)
                            nc.tensor.matmul(out=ps_t[:], lhsT=lhsT, rhs=rs,
                                             start=(v0 == 0),
                                             stop=(v0 == nv0 - 1))
                return psA, psB

            def stage_back(j, psA, psB):
                rimg = ipool.tile([M, W], f32, tag="rimg")
                nimg = ipool.tile([M, C * W], f32, tag="nimg")
                nc.vector.reciprocal(out=rimg[:], in_=psB[:, W:2 * W])
                nimg3 = nimg[:].rearrange("p (c q) -> p c q", q=W)
                for c, (pt, off) in enumerate(((psA, 0), (psA, W), (psB, 0))):
                    nc.vector.tensor_tensor(
                        out=nimg3[:, c, :], in0=pt[:, off:off + W],
                        in1=rimg[:], op=mybir.AluOpType.mult)

                # unfold: out[dy, c, hi, dx, wi] = nimg[hi+dy, c, wi+dx].
                # Issued from the otherwise-idle GPSIMD queue so the DMA sem
                # waits don't head-of-line-block ACT/SP work of later tasks.
                nv = nimg[:]
                npitch = nv.ap[0][0]
                if do_out:
                    for dy in range(PS):
                        for c in range(C):
                            s = nimg[dy:dy + ROUT, :]
                            src = bass.AP(s.tensor, s.offset + c * W,
                                          [[npitch, ROUT], [1, PS], [1, NH]])
                            nc.gpsimd.dma_start(out=OUT[j, dy, c], in_=src)

            # software-pipelined emission: task j+1's loads/TTs/matmuls are
            # emitted before task j's normalize/unfold, so each engine's
            # in-order stream never stalls on the previous task's tail.
            loop_cm = (tc.For_i(0, loop_reps) if loop_reps > 1
                       else contextlib.nullcontext())
            with loop_cm:
                pend = None
                for j in range(NT):
                    h = stage_front(j)
                    if pend is not None:
                        stage_back(*pend)
                    pend = (j,) + h
                stage_back(*pend)
    nc.finalize()
    return nc


def _host_prep(x, nlDists):
    # xt[tau, hi, v, wi] = x[i, hi*156+wi, f, v],  tau = i*14+f
    xt = np.ascontiguousarray(
        x.reshape(2, NH, NH, HORF, VF).transpose(0, 3, 1, 4, 2)
    ).reshape(T, NH, VF, NH)
    d6 = np.ascontiguousarray(nlDists[:, :, 0]).reshape(T, NH, NH)
    # each 156-wide feature plane is shipped as [4 zero cols | data] so the
    # device canvas loads with one contiguous DMA
    XSa = np.zeros((2 * T, RIN, VF, W), np.float32)
    DSa = np.empty((2 * T, RIN, NH), np.float32)
    XSa[0::2, :, :, 4:] = xt[:, :RIN]
    XSa[1::2, :, :, 4:] = xt[:, NH - RIN:][:, ::-1][:, :, VPERM, :]
    DSa[0::2] = d6[:, :RIN]
    DSa[1::2] = d6[:, NH - RIN:][:, ::-1]
    return (XSa.reshape(NCORES, NT, RIN, VF * W),
            DSa.reshape(NCORES, NT, RIN, NH))


def _host_post(OUTa):
    # OUTa: (8, 7, 5, 3, 78, 5, 156) -> (2, 24336, 14, 75)
    O = OUTa.reshape(2 * T, PS, C, ROUT, PS, NH)
    top, bot = O[0::2], O[1::2]
    out6 = np.empty((T, NH, NH, C, PS, PS), np.float32)
    # [tau, dy, c, hi, dx, wi] -> [tau, hi, wi, c, dy, dx]
    out6[:, :ROUT] = top.transpose(0, 3, 5, 2, 1, 4)
    out6[:, ROUT:] = bot[:, ::-1].transpose(0, 3, 5, 2, 1, 4)[:, ::-1]
    out_flat = out6.reshape(T, NPATCH, VF)
    final = out_flat.reshape(2, HORF, VF, NPATCH).transpose(0, 3, 1, 2)
    return np.ascontiguousarray(final)


def _is_self_inds(nlInds):
    k0 = np.asarray(nlInds)[:, :, 0, :]
    j = np.arange(NPATCH)
    return (bool((k0[:, :, 0] == np.arange(T, dtype=k0.dtype)[:, None]).all())
            and bool((k0[:, :, 1] == (j // NH).astype(k0.dtype)).all())
            and bool((k0[:, :, 2] == (j % NH).astype(k0.dtype)).all()))


def _numpy_fallback(x, nlDists, nlInds, H, Wp):
    images, patches, hor_f, ver_f = x.shape
    t = images * hor_f
    N = t * patches
    xr = np.transpose(x, (0, 2, 3, 1)).reshape(t, ver_f, patches)
    pat = np.transpose(xr, (0, 2, 1)).reshape(N, C, PS, PS)
    w = np.exp(-nlDists[:, :, 0].reshape(N))
    inds = nlInds[:, :, 0, :].reshape(N, 3)
    ti, hi, wi = inds[:, 0], inds[:, 1], inds[:, 2]
    d = np.arange(PS)
    sidx = (ti[:, None, None] * (H * Wp)
            + (hi[:, None, None] + d[None, :, None]) * Wp
            + (wi[:, None, None] + d[None, None, :])).reshape(-1)
    vals = (w[:, None, None, None] * pat).transpose(0, 2, 3, 1).reshape(-1, C)
    img = np.zeros((t * H * Wp, C), x.dtype)
    np.add.at(img, sidx, vals)
    wimg = np.zeros((t * H * Wp,), x.dtype)
    np.add.at(wimg, sidx, np.repeat(w, PS * PS))
    img = img / wimg[:, None]
    out_pat = img[sidx].reshape(N, PS, PS, C).transpose(0, 3, 1, 2)
    out = out_pat.reshape(t, patches, ver_f)
    return np.ascontiguousarray(
        out.reshape(images, hor_f, ver_f, patches).transpose(0, 3, 1, 2))


def kernel(x, nlDists, nlInds, pixels_h, pixels_w):
    global LAST_EXEC_NS
    import os
    x = np.asarray(x, np.float32)
    nlDists = np.asarray(nlDists, np.float32)
    if (x.shape != (2, NPATCH, HORF, VF) or int(pixels_h) != 160
            or int(pixels_w) != 160 or not _is_self_inds(nlInds)):
        return _numpy_fallback(np.asarray(x), np.asarray(nlDists),
                               np.asarray(nlInds), int(pixels_h), int(pixels_w))

    from concourse.bass_utils import run_bass_kernel_spmd
    XSa, DSa = _host_prep(x, nlDists)
    nc = _build_program()
    in_maps = [{"xs": XSa[c], "ds": DSa[c]} for c in range(NCORES)]
    trace = bool(os.environ.get("BASS_KERNEL_PROFILE"))
    res = run_bass_kernel_spmd(nc, in_maps, list(range(NCORES)), trace=trace)
    LAST_EXEC_NS = res.exec_time_ns
    OUTa = np.stack([np.asarray(res.results[c]["out"], np.float32)
                     for c in range(NCORES)])
    return _host_post(OUTa)
